# revision 1
# baseline (speedup 1.0000x reference)
"""Llama GQA chunk-attention layer on 8 Trainium2 NeuronCores.

Tensor-parallel over heads: core c computes Q heads [4c, 4c+4), KV head c,
and the partial output (attn_out_c @ Wo[rows of c's heads]); the host sums
the 8 partials.

Shapes (hardcoded): B=2, S=2048, HIDDEN=4096, 32 Q heads, 8 KV heads, D=128.
"""

import math
from contextlib import ExitStack

import ml_dtypes
import numpy as np

import concourse.bass as bass
import concourse.mybir as mybir
import concourse.tile as tile
from concourse.bass_utils import run_bass_kernel_spmd

# Problem constants
B, S, HID = 2, 2048, 4096
T = B * S                  # 4096 tokens
NH, NKV, D = 32, 8, 128
NCORES = 8
NH_C = NH // NCORES        # 4 q heads per core
DQ_C = NH_C * D            # 512
ROPE_BASE = 10000.0
SCALE = D ** -0.5

P = 128                    # partitions
TOKBLK = 256               # X^T chunk width for projections
NTOKBLK = T // TOKBLK      # 16
HO = HID // P              # 32 hidden 128-tiles
NTT = T // P               # 32 token 128-tiles
QBLK = 512                 # attention q-block width
NQB = S // QBLK            # 4 q-blocks per batch
KT_PER_B = S // P          # 16 k-tiles per batch

BF16 = mybir.dt.bfloat16
F32 = mybir.dt.float32


def build_bass():
    nc = bass.Bass()

    xt = nc.dram_tensor("xt", [HID, T], BF16, kind="ExternalInput")
    wq = nc.dram_tensor("wq", [HID, DQ_C], BF16, kind="ExternalInput")
    wk = nc.dram_tensor("wk", [HID, D], BF16, kind="ExternalInput")
    wv = nc.dram_tensor("wv", [HID, D], BF16, kind="ExternalInput")
    wo = nc.dram_tensor("wo", [DQ_C, HID], BF16, kind="ExternalInput")
    cos = nc.dram_tensor("cos", [D // 2, T], BF16, kind="ExternalInput")
    sin = nc.dram_tensor("sin", [D // 2, T], BF16, kind="ExternalInput")
    maskc = nc.dram_tensor("maskc", [P, 2 * QBLK], BF16, kind="ExternalInput")
    negd = nc.dram_tensor("negd", [P, P], BF16, kind="ExternalInput")
    ones = nc.dram_tensor("ones", [P, 1], BF16, kind="ExternalInput")
    out = nc.dram_tensor("out", [T, HID], F32, kind="ExternalOutput")
    rbounce = nc.dram_tensor("rbounce", [B * NH_C * NQB, QBLK], F32)

    xt_r = xt.rearrange("(ho p) t -> p ho t", p=P)

    with tile.TileContext(nc) as tc, ExitStack() as ctx:
        singles = ctx.enter_context(tc.tile_pool(name="singles", bufs=1))
        xpool = ctx.enter_context(tc.tile_pool(name="xpool", bufs=2))
        tmps = ctx.enter_context(tc.tile_pool(name="tmps", bufs=2))
        ptpool = ctx.enter_context(tc.tile_pool(name="ptpool", bufs=4))
        rpool = ctx.enter_context(tc.tile_pool(name="rpool", bufs=3))
        opool = ctx.enter_context(tc.tile_pool(name="opool", bufs=3))
        ps_acc = ctx.enter_context(tc.tile_pool(name="ps_acc", bufs=2, space="PSUM"))
        ps_st = ctx.enter_context(tc.tile_pool(name="ps_st", bufs=4, space="PSUM"))
        ps_sum = ctx.enter_context(tc.tile_pool(name="ps_sum", bufs=2, space="PSUM"))

        # ---- resident SBUF tensors ----
        wq_sb = singles.tile([P, HO, DQ_C], BF16, tag="bigw")  # shared slot with wo
        wk_sb = singles.tile([P, HO, D], BF16, tag="wk")
        wv_sb = singles.tile([P, HO, D], BF16, tag="wv")
        qT_sb = singles.tile([P, NH_C, T], BF16, tag="qT")
        kT_sb = singles.tile([P, T], BF16, tag="kT")
        v_sb = singles.tile([P, NTT, D], BF16, tag="v")
        aT_sb = singles.tile([P, NH_C, T], BF16, tag="aT")     # attn_out^T
        cos_sb = singles.tile([D // 2, T], BF16, tag="cos")
        sin_sb = singles.tile([D // 2, T], BF16, tag="sin")
        maskc_sb = singles.tile([P, 2 * QBLK], BF16, tag="maskc")
        negd_sb = singles.tile([P, P], BF16, tag="negd")
        ones_sb = singles.tile([P, 1], BF16, tag="ones")
        scratch = singles.tile([1, 4], F32, tag="scratch")

        nc.sync.dma_start(wq_sb, wq.rearrange("(ho p) m -> p ho m", p=P))
        nc.sync.dma_start(wk_sb, wk.rearrange("(ho p) m -> p ho m", p=P))
        nc.sync.dma_start(wv_sb, wv.rearrange("(ho p) m -> p ho m", p=P))
        nc.sync.dma_start(cos_sb, cos[:, :])
        nc.sync.dma_start(sin_sb, sin[:, :])
        nc.sync.dma_start(maskc_sb, maskc[:, :])
        nc.sync.dma_start(negd_sb, negd[:, :])
        nc.sync.dma_start(ones_sb, ones[:, :])

        def rope(dst_lo, dst_hi, src_psum, t0):
            # dst = [src_lo*cos - src_hi*sin ; src_hi*cos + src_lo*sin]
            c_sl = cos_sb[:, t0:t0 + TOKBLK]
            s_sl = sin_sb[:, t0:t0 + TOKBLK]
            h = D // 2
            # 1-element copy absorbs the PE wait so the tensor_tensor ops
            # below carry <=1 sync wait (TT ISA slot limit).
            nc.vector.tensor_copy(scratch[0:1, 0:1], src_psum[0:1, 0:1])
            ta = tmps.tile([h, TOKBLK], F32, tag="ta")
            tb = tmps.tile([h, TOKBLK], F32, tag="tb")
            nc.vector.tensor_mul(ta, src_psum[0:h, :], c_sl)
            nc.vector.tensor_mul(tb, src_psum[h:P, :], s_sl)
            nc.vector.tensor_sub(dst_lo, ta, tb)
            ta2 = tmps.tile([h, TOKBLK], F32, tag="ta")
            tb2 = tmps.tile([h, TOKBLK], F32, tag="tb")
            nc.vector.tensor_mul(ta2, src_psum[h:P, :], c_sl)
            nc.vector.tensor_mul(tb2, src_psum[0:h, :], s_sl)
            nc.vector.tensor_add(dst_hi, ta2, tb2)

        # ---- phase 1: projections + RoPE ----
        for tb_i in range(NTOKBLK):
            t0 = tb_i * TOKBLK
            xt_sb = xpool.tile([P, HO, TOKBLK], BF16, tag="xt")
            nc.sync.dma_start(xt_sb, xt_r[:, :, t0:t0 + TOKBLK])

            # Q^T [dq, tok] per head tile; rope; -> qT_sb
            for hq in range(NH_C):
                psq = ps_st.tile([P, TOKBLK], F32, tag="st")
                for ho in range(HO):
                    nc.tensor.matmul(
                        psq,
                        lhsT=wq_sb[:, ho, hq * P:(hq + 1) * P],
                        rhs=xt_sb[:, ho, :],
                        start=(ho == 0), stop=(ho == HO - 1),
                    )
                rope(qT_sb[0:D // 2, hq, t0:t0 + TOKBLK],
                     qT_sb[D // 2:P, hq, t0:t0 + TOKBLK], psq, t0)

            # K^T [dk, tok]; rope; -> kT_sb
            psk = ps_st.tile([P, TOKBLK], F32, tag="st")
            for ho in range(HO):
                nc.tensor.matmul(
                    psk, lhsT=wk_sb[:, ho, :], rhs=xt_sb[:, ho, :],
                    start=(ho == 0), stop=(ho == HO - 1),
                )
            rope(kT_sb[0:D // 2, t0:t0 + TOKBLK],
                 kT_sb[D // 2:P, t0:t0 + TOKBLK], psk, t0)

            # V [tok, dv] -> v_sb
            for tt in range(TOKBLK // P):
                g = (t0 // P) + tt
                psv = ps_st.tile([P, D], F32, tag="st")
                for ho in range(HO):
                    nc.tensor.matmul(
                        psv,
                        lhsT=xt_sb[:, ho, tt * P:(tt + 1) * P],
                        rhs=wv_sb[:, ho, :],
                        start=(ho == 0), stop=(ho == HO - 1),
                    )
                nc.any.tensor_copy(v_sb[:, g, :], psv)

        # ---- phase 2: causal attention (transposed orientation) ----
        for b in range(B):
            for hq in range(NH_C):
                for qb in range(NQB):
                    q0 = b * S + qb * QBLK
                    nkt = (qb + 1) * (QBLK // P)
                    ps_av = ps_acc.tile([P, QBLK], F32, tag="acc")
                    ps_rs = ps_sum.tile([1, QBLK], F32, tag="sum")
                    for kt in range(nkt):
                        gk = b * KT_PER_B + kt
                        pst = ps_st.tile([P, QBLK], F32, tag="st")
                        j = kt - qb * (QBLK // P)
                        diag = j >= 0
                        nc.tensor.matmul(
                            pst,
                            lhsT=kT_sb[:, gk * P:(gk + 1) * P],
                            rhs=qT_sb[:, hq, q0:q0 + QBLK],
                            start=True, stop=not diag,
                        )
                        if diag:
                            # accumulate -60000 on causally-invalid entries
                            off = QBLK - j * P
                            nc.tensor.matmul(
                                pst, lhsT=negd_sb[:, :],
                                rhs=maskc_sb[:, off:off + QBLK],
                                start=False, stop=True,
                            )
                        pT = ptpool.tile([P, QBLK], BF16, tag="pt")
                        nc.scalar.activation(
                            pT, pst, mybir.ActivationFunctionType.Exp)
                        nc.tensor.matmul(
                            ps_rs, lhsT=ones_sb[:, 0:1], rhs=pT,
                            start=(kt == 0), stop=(kt == nkt - 1),
                        )
                        nc.tensor.matmul(
                            ps_av, lhsT=v_sb[:, gk, :], rhs=pT,
                            start=(kt == 0), stop=(kt == nkt - 1),
                        )
                    r = rpool.tile([1, QBLK], F32, tag="r")
                    nc.vector.reciprocal(r, ps_rs)
                    # partition-broadcast via DRAM bounce (SBUF APs need
                    # nonzero partition step; DRAM APs don't)
                    row = (b * NH_C + hq) * NQB + qb
                    nc.sync.dma_start(rbounce[row:row + 1, :], r)
                    r_bc = rpool.tile([P, QBLK], F32, tag="rbc")
                    nc.sync.dma_start(
                        r_bc, rbounce[row:row + 1, :].to_broadcast([P, QBLK]))
                    # absorb the gpsimd wait so the normalize TT has <=1 wait
                    nc.vector.tensor_copy(scratch[0:1, 1:2], r_bc[0:1, 0:1])
                    nc.vector.tensor_mul(
                        aT_sb[:, hq, q0:q0 + QBLK], ps_av, r_bc)

        # ---- phase 3: output projection (partial) ----
        wo_sb = singles.tile([P, NH_C, HID], BF16, tag="bigw")
        nc.sync.dma_start(wo_sb, wo.rearrange("(h p) n -> p h n", p=P))
        for tt in range(NTT):
            for hb in range(HID // QBLK):
                pso = ps_acc.tile([P, QBLK], F32, tag="acc")
                for hq in range(NH_C):
                    nc.tensor.matmul(
                        pso,
                        lhsT=aT_sb[:, hq, tt * P:(tt + 1) * P],
                        rhs=wo_sb[:, hq, hb * QBLK:(hb + 1) * QBLK],
                        start=(hq == 0), stop=(hq == NH_C - 1),
                    )
                o_t = opool.tile([P, QBLK], F32, tag="ot")
                nc.any.tensor_copy(o_t, pso)
                nc.sync.dma_start(
                    out[tt * P:(tt + 1) * P, hb * QBLK:(hb + 1) * QBLK], o_t)

    return nc


def _prep_inputs(hidden_states, position_ids, Wq, Wk, Wv, Wo):
    bf16 = ml_dtypes.bfloat16
    hs = np.asarray(hidden_states, np.float32).reshape(T, HID)
    xt = np.ascontiguousarray(hs.T).astype(bf16)

    pos = np.asarray(position_ids, np.int64).reshape(B, S)
    inv_freq = 1.0 / (ROPE_BASE ** (np.arange(0, D, 2, dtype=np.float32) / D))
    ang = pos.astype(np.float32)[:, :, None] * inv_freq[None, None, :]  # [B,S,64]
    cosT = np.ascontiguousarray(
        np.cos(ang).reshape(T, D // 2).T).astype(bf16)  # [64, T]
    sinT = np.ascontiguousarray(
        np.sin(ang).reshape(T, D // 2).T).astype(bf16)

    Wq = np.asarray(Wq, np.float32) * SCALE  # fold softmax scale into Q
    Wk = np.asarray(Wk, np.float32)
    Wv = np.asarray(Wv, np.float32)
    Wo = np.asarray(Wo, np.float32)

    # maskc[kk, x] = 1 where the (shifted) causal position is invalid:
    # sliced as maskc[:, QBLK - j*P :][:, :QBLK], giving 1 iff kk > qq - j*P
    x = np.arange(2 * QBLK)[None, :]
    maskc = (np.arange(P)[:, None] > (x - QBLK)).astype(bf16)
    negd = (np.eye(P, dtype=np.float32) * -60000.0).astype(bf16)
    ones = np.ones((P, 1), bf16)

    shared = {"xt": xt, "cos": cosT, "sin": sinT, "maskc": maskc,
              "negd": negd, "ones": ones}
    in_maps = []
    for c in range(NCORES):
        m = dict(shared)
        m["wq"] = np.ascontiguousarray(
            Wq[:, c * DQ_C:(c + 1) * DQ_C]).astype(bf16)
        m["wk"] = np.ascontiguousarray(Wk[:, c * D:(c + 1) * D]).astype(bf16)
        m["wv"] = np.ascontiguousarray(Wv[:, c * D:(c + 1) * D]).astype(bf16)
        m["wo"] = np.ascontiguousarray(
            Wo[c * DQ_C:(c + 1) * DQ_C, :]).astype(bf16)
        in_maps.append(m)
    return in_maps


_LAST_EXEC_NS = None


def legalize_sync_waits(js: bytes) -> bytes:
    """This walrus accepts at most one embedded sync wait per instruction.
    Hoist extra waits (and extra updates) onto standalone EventSemaphore
    instructions inserted just before (waits) / after (updates) on the same
    engine stream."""
    import json
    d = json.loads(js)
    n_new = 0
    for fn in d["functions"]:
        for blk in fn["blocks"]:
            out = []
            for inst in blk["instructions"]:
                si = inst.get("sync_info")
                waits = (si or {}).get("on_wait") or []
                updates = (si or {}).get("on_update") or []
                if len(waits) > 1:
                    for w in waits[:-1]:
                        n_new += 1
                        out.append({
                            "debug": inst.get("debug", 0),
                            "engine": inst["engine"],
                            "ins": [], "outs": [],
                            "name": f"{inst['name']}-hw{n_new}",
                            "opcode": "EventSemaphore",
                            "sync_info": {"on_wait": [w], "on_update": []},
                        })
                    si["on_wait"] = [waits[-1]]
                out.append(inst)
                if len(updates) > 1:
                    for u in updates[1:]:
                        n_new += 1
                        out.append({
                            "debug": inst.get("debug", 0),
                            "engine": inst["engine"],
                            "ins": [], "outs": [],
                            "name": f"{inst['name']}-hu{n_new}",
                            "opcode": "EventSemaphore",
                            "sync_info": {"on_wait": [], "on_update": [u]},
                        })
                    si["on_update"] = [updates[0]]
            blk["instructions"] = out
    return json.dumps(d).encode()


def _run(in_maps, trace=False):
    global _LAST_EXEC_NS
    nc = build_bass()
    legalized = legalize_sync_waits(nc.to_json_bytes())
    nc.to_json_bytes = lambda: legalized
    res = run_bass_kernel_spmd(nc, in_maps, core_ids=list(range(NCORES)),
                               trace=trace)
    _LAST_EXEC_NS = res.exec_time_ns
    return res.results


def kernel(hidden_states, position_ids, Wq, Wk, Wv, Wo):
    import os
    in_maps = _prep_inputs(hidden_states, position_ids, Wq, Wk, Wv, Wo)
    trace = bool(int(os.environ.get("KERNEL_TRACE", "0")))
    results = _run(in_maps, trace=trace)
    total = np.zeros((T, HID), np.float64)
    for r in results:
        total += r["out"].astype(np.float64)
    return total.astype(np.float32).reshape(B, S, HID)

